# revision 6
# baseline (speedup 1.0000x reference)
"""Trainium2 Bass kernel for a 2-layer GCN encoder (GCNConv x2 + global mean pool).

Math: with A' = A + I and deg = indegree(A') (symmetric-norm GCN),
    gcn(h, W, b) = D^-1/2 A' D^-1/2 (h) W + b
factorized as  out = dinv * (A' @ (dinv * h)) @ W + b   (dinv = deg^-1/2)
so the SpMM is pure 0/1 structure; per-edge norms become per-node row scales.

Sharding: dst-node ranges across 8 cores (6272 rows / 49 tiles each). The
per-edge source-row gather uses gpsimd `dma_gather` (one SWDGE op covers
thousands of rows at ~0.34ns/descriptor) instead of per-128-row indirect
DMAs (994ns fixed each) — that fixed overhead was the previous bottleneck.
dma_gather indices are int16, so the (replicated) HBM feature table is
addressed as two 25088-row halves; each dst tile's edge list (self-loops
included as ordinary indices) is split by source half. Tiles are processed
in groups of 7: two dma_gathers per group (low/high half) land all chunks
in SBUF in the [slot%128 -> partition, slot//128 -> chunk] layout the
scatter expects. Per tile, a bf16 0/1 scatter matrix (dst-offset vs iota
compare on VectorE) feeds TensorE one-hot matmuls accumulating psumT
[feat, dstoff] in PSUM, then the dense W matmul + rank-1 bias matmul +
fused relu/dinv scaling as before. Two SPMD launches (layer 1 -> host
allgather of the 1.6MB/core slabs -> layer 2 + graph pooling).
"""
import math
import numpy as np
import ml_dtypes

from concourse import bass, mybir, tile, bacc
from concourse.bass_utils import run_bass_kernel_spmd
from concourse._compat import get_trn_type

N_CORES = 8
P = 128          # partitions / tile rows
D = 128          # feature dim
G = 512          # number of graphs (fixed by the problem)
GROUP = 7        # dst tiles per gather group (49 = 7*7)
F32 = mybir.dt.float32
BF16 = mybir.dt.bfloat16
I16 = mybir.dt.int16

USE_BF16 = True     # bf16 gather table (half the random-gather bytes; rel err ~1e-4)


# ---------------------------------------------------------------- host prep

def _pad_chunks(idx, off, nchunks):
    """Pad (idx, off) to nchunks*P slots: idx pads with 0 (valid row,
    masked by off=P in the scatter matrix)."""
    n = nchunks * P
    ip = np.zeros(n, dtype=np.int16)
    ip[:len(idx)] = idx
    op = np.full(n, float(P), dtype=np.float32)
    op[:len(off)] = off
    return ip, op


def preprocess(x, edge_index, batch):
    N = x.shape[0]
    rows_per_core = int(math.ceil(N / (N_CORES * P))) * P
    npad = rows_per_core * N_CORES
    tiles = rows_per_core // P
    n_tiles_g = N_CORES * tiles
    H = npad // 2
    assert H % P == 0 and rows_per_core % P == 0
    ngroups = (tiles + GROUP - 1) // GROUP

    src = edge_index[0].astype(np.int64)
    dst = edge_index[1].astype(np.int64)
    deg = (np.bincount(dst, minlength=N) + 1).astype(np.float32)
    dinv = 1.0 / np.sqrt(np.maximum(deg, 1.0))

    xhat = np.zeros((npad, D), dtype=np.float32)
    xhat[:N] = x.astype(np.float32) * dinv[:, None]

    order = np.argsort(dst)
    src_s = src[order]
    dst_s = dst[order]
    bounds = np.searchsorted(dst_s, np.arange(0, npad + 1, P))

    # Per (core, tile): edge + self-loop source lists split by table half.
    lo_lists = [[None] * tiles for _ in range(N_CORES)]
    hi_lists = [[None] * tiles for _ in range(N_CORES)]
    selfid = np.arange(P, dtype=np.int64)
    for k in range(N_CORES):
        for t in range(tiles):
            gt = k * tiles + t
            s, e = int(bounds[gt]), int(bounds[gt + 1])
            asrc = np.concatenate([src_s[s:e], gt * P + selfid])
            aoff = np.concatenate([(dst_s[s:e] - gt * P).astype(np.float32),
                                   selfid.astype(np.float32)])
            m = asrc < H
            lo_lists[k][t] = (asrc[m].astype(np.int16), aoff[m])
            hi_lists[k][t] = ((asrc[~m] - H).astype(np.int16), aoff[~m])

    c_lo = np.array([max(-(-len(lo_lists[k][t][0]) // P) for k in range(N_CORES))
                     for t in range(tiles)], dtype=np.int64)
    c_hi = np.array([max(-(-len(hi_lists[k][t][0]) // P) for k in range(N_CORES))
                     for t in range(tiles)], dtype=np.int64)

    # Static group metadata (identical across cores -> SPMD-uniform program).
    groups = []          # per group: dict with tile list + chunk bookkeeping
    sum_ca = int((c_lo + c_hi).sum())
    cg_max = 0
    ct_max = 0
    tile_col = np.zeros(tiles + 1, dtype=np.int64)   # dstoff col offset, tile-major
    for t in range(tiles):
        tile_col[t + 1] = tile_col[t] + c_lo[t] + c_hi[t]
        ct_max = max(ct_max, int(c_lo[t] + c_hi[t]))
    gather_chunk_off = 0     # running chunk offset in gather order
    for g0 in range(0, tiles, GROUP):
        ts = list(range(g0, min(g0 + GROUP, tiles)))
        Lg = int(sum(c_lo[t] for t in ts))
        Hg = int(sum(c_hi[t] for t in ts))
        lo_start = {}
        hi_start = {}
        acc = 0
        for t in ts:
            lo_start[t] = acc
            acc += int(c_lo[t])
        acc = 0
        for t in ts:
            hi_start[t] = acc
            acc += int(c_hi[t])
        groups.append(dict(tiles=ts, Lg=Lg, Hg=Hg, lo_start=lo_start,
                           hi_start=hi_start, chunk_off=gather_chunk_off))
        gather_chunk_off += Lg + Hg
        cg_max = max(cg_max, Lg + Hg)
    assert gather_chunk_off == sum_ca

    # Per-core flat streams.
    #  - idx stream in GATHER order: per group, lo chunks (tile-major) then hi.
    #  - dstoff in TILE-major order: per tile, lo chunks then hi chunks.
    idx_w = np.zeros((N_CORES, P, sum_ca * (P // 16)), dtype=np.int16)
    dstoff = np.full((N_CORES, P, sum_ca), float(P), dtype=np.float32)
    for k in range(N_CORES):
        stream = []
        for g in groups:
            for t in g['tiles']:
                ip, _ = _pad_chunks(*lo_lists[k][t], int(c_lo[t]))
                stream.append(ip)
            for t in g['tiles']:
                ip, _ = _pad_chunks(*hi_lists[k][t], int(c_hi[t]))
                stream.append(ip)
        stream = np.concatenate(stream)
        assert stream.shape[0] == sum_ca * P
        idx_w[k] = np.tile(stream.reshape(-1, 16).T, (8, 1))
        for t in range(tiles):
            col = int(tile_col[t])
            _, op_ = _pad_chunks(*lo_lists[k][t], int(c_lo[t]))
            if c_lo[t]:
                dstoff[k, :, col:col + int(c_lo[t])] = op_.reshape(-1, P).T
            _, op_ = _pad_chunks(*hi_lists[k][t], int(c_hi[t]))
            if c_hi[t]:
                dstoff[k, :, col + int(c_lo[t]):col + int(c_lo[t] + c_hi[t])] = \
                    op_.reshape(-1, P).T

    dinv_pad = np.zeros(npad, dtype=np.float32)
    dinv_pad[:N] = dinv
    dinv_slab = dinv_pad.reshape(N_CORES, tiles, P).transpose(0, 2, 1).copy()
    sdeg_pad = np.zeros(npad, dtype=np.float32)
    sdeg_pad[:N] = np.sqrt(np.maximum(deg, 1.0))
    sdeg_rows = sdeg_pad.reshape(N_CORES, 1, tiles * P).copy()

    batch_pad = np.full(npad, -1, dtype=np.int64)
    batch_pad[:N] = batch.astype(np.int64)
    g0s = np.zeros(N_CORES, dtype=np.int64)
    batchoff = np.full((N_CORES, P, tiles), float(P), dtype=np.float32)
    for k in range(N_CORES):
        b = batch_pad[k * rows_per_core:(k + 1) * rows_per_core]
        real = b >= 0
        assert real.any()
        g0s[k] = b[real].min()
        span = int(b[real].max() - g0s[k]) + 1
        assert span <= P - 1, f"graph span {span} exceeds pooling tile"
        off = np.full(rows_per_core, float(P), dtype=np.float32)
        off[real] = (b[real] - g0s[k]).astype(np.float32)
        batchoff[k] = off.reshape(tiles, P).T

    iota = np.tile(np.arange(P, dtype=np.float32), (P, ct_max))
    cnt_g = np.bincount(batch.astype(np.int64), minlength=G).astype(np.float32)

    return dict(N=N, npad=npad, H=H, rows_per_core=rows_per_core,
                tiles_per_core=tiles, c_lo=c_lo, c_hi=c_hi, sum_ca=sum_ca,
                cg_max=cg_max, ct_max=ct_max, tile_col=tile_col,
                groups=groups, idx_w=idx_w, dstoff=dstoff,
                dinv_slab=dinv_slab, sdeg_rows=sdeg_rows,
                batchoff=batchoff, g0=g0s, iota=iota, xhat=xhat, cnt_g=cnt_g)


# ---------------------------------------------------------------- device

def build_layer(pre, last_layer: bool, reps: int = 1, bf16_table: bool = True):
    """One SPMD program: grouped dma_gather + SpMM + dense matmul per dst tile.
    last_layer=False: hhat slab [rows_per_core, D] = dinv*relu(dinv*z)
    last_layer=True:  pooled [P, D] = sum over graph-offset of dinv*z
    """
    tiles = pre['tiles_per_core']
    c_lo = pre['c_lo']
    c_hi = pre['c_hi']
    sum_ca = pre['sum_ca']
    cg_max = pre['cg_max']
    ct_max = pre['ct_max']
    tile_col = pre['tile_col']
    groups = pre['groups']
    npad = pre['npad']
    H = pre['H']

    TDT = BF16 if bf16_table else F32
    nc = bacc.Bacc(get_trn_type() or "TRN2", target_bir_lowering=False, debug=False)
    table = nc.dram_tensor("table", [npad, D], TDT, kind="ExternalInput").ap()
    idx = nc.dram_tensor("idx", [P, sum_ca * (P // 16)], I16, kind="ExternalInput").ap()
    dstoff = nc.dram_tensor("dstoff", [P, sum_ca], BF16, kind="ExternalInput").ap()
    iota = nc.dram_tensor("iota", [P, ct_max * P], BF16, kind="ExternalInput").ap()
    Wt = nc.dram_tensor("W", [D, D], F32, kind="ExternalInput").ap()
    bt = nc.dram_tensor("b", [1, D], F32, kind="ExternalInput").ap()
    dinv = nc.dram_tensor("dinv", [P, tiles], F32, kind="ExternalInput").ap()
    sdeg = nc.dram_tensor("sdeg", [1, tiles * P], F32, kind="ExternalInput").ap()
    if last_layer:
        batchoff = nc.dram_tensor("batchoff", [P, tiles], BF16, kind="ExternalInput").ap()
        pooled = nc.dram_tensor("pooled", [P, D], F32, kind="ExternalOutput").ap()
    else:
        hhat = nc.dram_tensor("hhat", [tiles * P, D], TDT, kind="ExternalOutput").ap()

    with tile.TileContext(nc) as tc:
        with tc.tile_pool(name="const", bufs=1) as cp, \
             tc.tile_pool(name="gather", bufs=2) as gp, \
             tc.tile_pool(name="sel", bufs=3) as sp, \
             tc.tile_pool(name="small", bufs=3) as mp, \
             tc.tile_pool(name="hh", bufs=2) as hp, \
             tc.tile_pool(name="ps1", bufs=2, space="PSUM") as pp1, \
             tc.tile_pool(name="ps2", bufs=2, space="PSUM") as pp2:
            idx_t = cp.tile([P, sum_ca * (P // 16)], I16)
            dst_t = cp.tile([P, sum_ca], BF16)
            iota_t = cp.tile([P, ct_max * P], BF16)
            W_t = cp.tile([D, D], F32)
            b_t = cp.tile([1, D], F32)
            sdeg_t = cp.tile([1, tiles * P], F32)
            dinv_t = cp.tile([P, tiles], F32)
            nc.sync.dma_start(out=idx_t[:], in_=idx[:])
            nc.sync.dma_start(out=dst_t[:], in_=dstoff[:])
            nc.sync.dma_start(out=iota_t[:], in_=iota[:])
            nc.sync.dma_start(out=W_t[:], in_=Wt[:])
            nc.sync.dma_start(out=b_t[:], in_=bt[:])
            nc.sync.dma_start(out=sdeg_t[:], in_=sdeg[:])
            nc.sync.dma_start(out=dinv_t[:], in_=dinv[:])
            if last_layer:
                boff_t = cp.tile([P, tiles], BF16)
                nc.sync.dma_start(out=boff_t[:], in_=batchoff[:])
                pooled_sb = cp.tile([P, D], F32)

            # one scalar reg per distinct gather length (reused across reps)
            reg_cache = {}
            for grp in groups:
                for n in (grp['Lg'] * P, grp['Hg'] * P):
                    if n and n not in reg_cache:
                        reg_cache[n] = nc.gpsimd.to_reg(n)

            for rep in range(reps):
                if last_layer:
                    nc.vector.memset(pooled_sb[:], 0.0)
                for gi, grp in enumerate(groups):
                    ts = grp['tiles']
                    Lg, Hg = grp['Lg'], grp['Hg']
                    co = grp['chunk_off']
                    g = gp.tile([P, cg_max * D], TDT, tag="g")
                    if Lg:
                        nc.gpsimd.dma_gather(
                            g[:, :Lg * D].rearrange("p (c j) -> p c j", j=D),
                            table[0:H, :],
                            idx_t[:, co * 8:(co + Lg) * 8],
                            Lg * P, reg_cache[Lg * P], D,
                            single_packet=False)
                    if Hg:
                        nc.gpsimd.dma_gather(
                            g[:, Lg * D:(Lg + Hg) * D].rearrange("p (c j) -> p c j", j=D),
                            table[H:npad, :],
                            idx_t[:, (co + Lg) * 8:(co + Lg + Hg) * 8],
                            Hg * P, reg_cache[Hg * P], D,
                            single_packet=False)
                    if not last_layer:
                        hh_g = hp.tile([P, len(ts) * D], TDT, tag="hh")
                    for ti, t in enumerate(ts):
                        nlo, nhi = int(c_lo[t]), int(c_hi[t])
                        ct = nlo + nhi
                        col = int(tile_col[t])
                        S = sp.tile([P, ct_max * D], BF16, tag="s")
                        nc.vector.tensor_tensor(
                            out=S[:, :ct * D].rearrange("p (c j) -> p c j", j=D),
                            in0=dst_t[:, col:col + ct].to_broadcast([P, ct, D]),
                            in1=iota_t[:, :ct * D].rearrange("p (c j) -> p c j", j=D),
                            op=mybir.AluOpType.is_equal)
                        psumT = pp1.tile([P, D], F32, space="PSUM", tag="pT")
                        gchunks = ([grp['lo_start'][t] + c for c in range(nlo)] +
                                   [Lg + grp['hi_start'][t] + c for c in range(nhi)])
                        for ci, gc in enumerate(gchunks):
                            nc.tensor.matmul(out=psumT[:],
                                             lhsT=g[:, gc * D:(gc + 1) * D],
                                             rhs=S[:, ci * D:(ci + 1) * D],
                                             start=(ci == 0), stop=(ci == ct - 1))
                        lhs_sb = mp.tile([P, D], F32, tag="lhs")
                        nc.vector.tensor_copy(out=lhs_sb[:], in_=psumT[:])
                        psum2 = pp2.tile([P, D], F32, space="PSUM", tag="p2")
                        nc.tensor.matmul(out=psum2[:], lhsT=lhs_sb[:], rhs=W_t[:],
                                         start=True, stop=False)
                        nc.tensor.matmul(out=psum2[:],
                                         lhsT=sdeg_t[:, t * P:(t + 1) * P],
                                         rhs=b_t[:], start=False, stop=True)
                        if last_layer:
                            out_sb = mp.tile([P, D], F32, tag="out")
                            nc.scalar.activation(out=out_sb[:], in_=psum2[:],
                                                 func=mybir.ActivationFunctionType.Copy,
                                                 scale=dinv_t[:, t:t + 1])
                            Pt = sp.tile([P, D], F32, tag="pool_sel")
                            nc.vector.tensor_tensor(
                                out=Pt[:],
                                in0=boff_t[:, t:t + 1].to_broadcast([P, D]),
                                in1=iota_t[:, :D],
                                op=mybir.AluOpType.is_equal)
                            pool_ps = pp2.tile([P, D], F32, space="PSUM", tag="pool_ps")
                            nc.tensor.matmul(out=pool_ps[:], lhsT=Pt[:], rhs=out_sb[:],
                                             start=True, stop=True)
                            nc.vector.tensor_add(out=pooled_sb[:], in0=pooled_sb[:],
                                                 in1=pool_ps[:])
                        else:
                            out_sb = mp.tile([P, D], F32, tag="out")
                            nc.scalar.activation(out=out_sb[:], in_=psum2[:],
                                                 func=mybir.ActivationFunctionType.Relu,
                                                 scale=dinv_t[:, t:t + 1])
                            nc.vector.tensor_scalar_mul(
                                out=hh_g[:, ti * D:(ti + 1) * D], in0=out_sb[:],
                                scalar1=dinv_t[:, t:t + 1])
                    if not last_layer:
                        t0 = ts[0]
                        nc.sync.dma_start(
                            out=hhat[t0 * P:(t0 + len(ts)) * P, :]
                                .rearrange("(c p) j -> p c j", p=P),
                            in_=hh_g[:, :len(ts) * D].rearrange("p (c j) -> p c j", j=D))
                if last_layer:
                    nc.sync.dma_start(out=pooled[:], in_=pooled_sb[:])
    nc.compile()
    return nc


def _in_maps(pre, table_np, W, b, last_layer):
    maps = []
    for k in range(N_CORES):
        m = dict(table=table_np,
                 idx=pre['idx_w'][k],
                 dstoff=pre['dstoff'][k].astype(ml_dtypes.bfloat16),
                 iota=pre['iota'].astype(ml_dtypes.bfloat16),
                 W=np.ascontiguousarray(W, dtype=np.float32),
                 b=np.ascontiguousarray(b, dtype=np.float32).reshape(1, D),
                 dinv=pre['dinv_slab'][k],
                 sdeg=pre['sdeg_rows'][k])
        if last_layer:
            m['batchoff'] = pre['batchoff'][k].astype(ml_dtypes.bfloat16)
        maps.append(m)
    return maps


def kernel(x, edge_index, batch, W1, b1, W2, b2):
    x = np.asarray(x); edge_index = np.asarray(edge_index)
    batch = np.asarray(batch)
    W1 = np.asarray(W1); b1 = np.asarray(b1)
    W2 = np.asarray(W2); b2 = np.asarray(b2)

    pre = preprocess(x, edge_index, batch)
    core_ids = list(range(N_CORES))

    tdt = ml_dtypes.bfloat16 if USE_BF16 else np.float32
    table1 = pre['xhat'].astype(tdt)
    nc1 = build_layer(pre, last_layer=False, bf16_table=USE_BF16)
    res1 = run_bass_kernel_spmd(nc1, _in_maps(pre, table1, W1, b1, False),
                                core_ids).results

    h1hat = np.zeros((pre['npad'], D), dtype=tdt)
    rpc = pre['rows_per_core']
    for k in range(N_CORES):
        h1hat[k * rpc:(k + 1) * rpc] = res1[k]['hhat']

    nc2 = build_layer(pre, last_layer=True, bf16_table=USE_BF16)
    res2 = run_bass_kernel_spmd(nc2, _in_maps(pre, h1hat, W2, b2, True),
                                core_ids).results

    pooled = np.zeros((G, D), dtype=np.float32)
    for k in range(N_CORES):
        part = res2[k]['pooled']
        g0 = int(pre['g0'][k])
        span = min(P, G - g0)
        pooled[g0:g0 + span] += part[:span]
    return pooled / np.maximum(pre['cnt_g'], 1.0)[:, None]


# revision 11
# speedup vs baseline: 2.6924x; 2.6924x over previous
"""Trainium2 Bass kernel for a 2-layer GCN encoder (GCNConv x2 + global mean pool).

Math: with A' = A + I and deg = indegree(A') (symmetric-norm GCN),
    gcn(h, W, b) = D^-1/2 A' D^-1/2 (h) W + b
factorized as  out = dinv * (A' @ (dinv * h)) @ W + b   (dinv = deg^-1/2)
so the SpMM is pure 0/1 structure; per-edge norms become per-node row scales.

Sharding: dst-node ranges across 8 cores (6272 rows / 49 tiles each). The
per-edge source-row gather uses gpsimd `dma_gather` (one SWDGE op covers
thousands of rows at ~0.34ns/descriptor) instead of per-128-row indirect
DMAs (994ns fixed each) — that fixed overhead was the previous bottleneck.
dma_gather indices are int16, so the (replicated) HBM feature table is
addressed as two 25088-row halves; each dst tile's edge list (self-loops
included as ordinary indices) is split by source half. Tiles are processed
in groups of 7: two dma_gathers per group (low/high half) land all chunks
in SBUF in the [slot%128 -> partition, slot//128 -> chunk] layout the
scatter expects. Per tile, a bf16 0/1 scatter matrix (dst-offset vs iota
compare on VectorE) feeds TensorE one-hot matmuls accumulating psumT
[feat, dstoff] in PSUM, then the dense W matmul + rank-1 bias matmul +
fused relu/dinv scaling as before. Two SPMD launches (layer 1 -> host
allgather of the 1.6MB/core slabs -> layer 2 + graph pooling).
"""
import math
import numpy as np
import ml_dtypes

from concourse import bass, mybir, tile, bacc
from concourse.bass_utils import run_bass_kernel_spmd
from concourse._compat import get_trn_type

N_CORES = 8
P = 128          # partitions / tile rows
D = 128          # feature dim
G = 512          # number of graphs (fixed by the problem)
GROUP = 7        # dst tiles per gather group (49 = 7*7)
NQ = 2           # SWDGE descriptor-generation queues
MAX_OP_CHUNKS = 35   # max 128-row chunks per dma_gather op (ring-safe)
F32 = mybir.dt.float32
BF16 = mybir.dt.bfloat16
I16 = mybir.dt.int16

USE_BF16 = True     # bf16 gather table (half the random-gather bytes; rel err ~1e-4)


# ---------------------------------------------------------------- host prep

def _pad_chunks(idx, off, nchunks):
    """Pad (idx, off) to nchunks*P slots: idx pads with 0 (valid row,
    masked by off=P in the scatter matrix)."""
    n = nchunks * P
    ip = np.zeros(n, dtype=np.int16)
    ip[:len(idx)] = idx
    op = np.full(n, float(P), dtype=np.float32)
    op[:len(off)] = off
    return ip, op


def preprocess(x, edge_index, batch):
    N = x.shape[0]
    rows_per_core = int(math.ceil(N / (N_CORES * P))) * P
    npad = rows_per_core * N_CORES
    tiles = rows_per_core // P
    n_tiles_g = N_CORES * tiles
    H = npad // 2
    assert H % P == 0 and rows_per_core % P == 0
    ngroups = (tiles + GROUP - 1) // GROUP

    src = edge_index[0].astype(np.int64)
    dst = edge_index[1].astype(np.int64)
    deg = (np.bincount(dst, minlength=N) + 1).astype(np.float32)
    dinv = 1.0 / np.sqrt(np.maximum(deg, 1.0))

    xhat = np.zeros((npad, D), dtype=np.float32)
    xhat[:N] = x.astype(np.float32) * dinv[:, None]

    order = np.argsort(dst)
    src_s = src[order]
    dst_s = dst[order]
    bounds = np.searchsorted(dst_s, np.arange(0, npad + 1, P))

    # Per (core, tile): edge + self-loop source lists split by table half.
    lo_lists = [[None] * tiles for _ in range(N_CORES)]
    hi_lists = [[None] * tiles for _ in range(N_CORES)]
    selfid = np.arange(P, dtype=np.int64)
    for k in range(N_CORES):
        for t in range(tiles):
            gt = k * tiles + t
            s, e = int(bounds[gt]), int(bounds[gt + 1])
            asrc = np.concatenate([src_s[s:e], gt * P + selfid])
            aoff = np.concatenate([(dst_s[s:e] - gt * P).astype(np.float32),
                                   selfid.astype(np.float32)])
            m = asrc < H
            lo_lists[k][t] = (asrc[m].astype(np.int16), aoff[m])
            hi_lists[k][t] = ((asrc[~m] - H).astype(np.int16), aoff[~m])

    c_lo = np.array([max(-(-len(lo_lists[k][t][0]) // P) for k in range(N_CORES))
                     for t in range(tiles)], dtype=np.int64)
    c_hi = np.array([max(-(-len(hi_lists[k][t][0]) // P) for k in range(N_CORES))
                     for t in range(tiles)], dtype=np.int64)

    # Static group metadata (identical across cores -> SPMD-uniform program).
    groups = []          # per group: dict with tile list + chunk bookkeeping
    sum_ca = int((c_lo + c_hi).sum())
    cg_max = 0
    ct_max = 0
    tile_col = np.zeros(tiles + 1, dtype=np.int64)   # dstoff col offset, tile-major
    for t in range(tiles):
        tile_col[t + 1] = tile_col[t] + c_lo[t] + c_hi[t]
        ct_max = max(ct_max, int(c_lo[t] + c_hi[t]))
    gather_chunk_off = 0     # running chunk offset in gather order
    for g0 in range(0, tiles, GROUP):
        ts = list(range(g0, min(g0 + GROUP, tiles)))
        Lg = int(sum(c_lo[t] for t in ts))
        Hg = int(sum(c_hi[t] for t in ts))
        lo_start = {}
        hi_start = {}
        acc = 0
        for t in ts:
            lo_start[t] = acc
            acc += int(c_lo[t])
        acc = 0
        for t in ts:
            hi_start[t] = acc
            acc += int(c_hi[t])
        groups.append(dict(tiles=ts, Lg=Lg, Hg=Hg, lo_start=lo_start,
                           hi_start=hi_start, chunk_off=gather_chunk_off))
        gather_chunk_off += Lg + Hg
        cg_max = max(cg_max, Lg + Hg)
    assert gather_chunk_off == sum_ca

    # Per-core flat streams.
    #  - idx stream in GATHER order: per group, lo chunks (tile-major) then hi.
    #  - dstoff in TILE-major order: per tile, lo chunks then hi chunks.
    idx_w = np.zeros((N_CORES, P, sum_ca * (P // 16)), dtype=np.int16)
    dstoff = np.full((N_CORES, P, sum_ca), float(P), dtype=np.float32)
    for k in range(N_CORES):
        stream = []
        for g in groups:
            for t in g['tiles']:
                ip, _ = _pad_chunks(*lo_lists[k][t], int(c_lo[t]))
                stream.append(ip)
            for t in g['tiles']:
                ip, _ = _pad_chunks(*hi_lists[k][t], int(c_hi[t]))
                stream.append(ip)
        stream = np.concatenate(stream)
        assert stream.shape[0] == sum_ca * P
        idx_w[k] = np.tile(stream.reshape(-1, 16).T, (8, 1))
        for t in range(tiles):
            col = int(tile_col[t])
            _, op_ = _pad_chunks(*lo_lists[k][t], int(c_lo[t]))
            if c_lo[t]:
                dstoff[k, :, col:col + int(c_lo[t])] = op_.reshape(-1, P).T
            _, op_ = _pad_chunks(*hi_lists[k][t], int(c_hi[t]))
            if c_hi[t]:
                dstoff[k, :, col + int(c_lo[t]):col + int(c_lo[t] + c_hi[t])] = \
                    op_.reshape(-1, P).T

    dinv_pad = np.zeros(npad, dtype=np.float32)
    dinv_pad[:N] = dinv
    dinv_slab = dinv_pad.reshape(N_CORES, tiles, P).transpose(0, 2, 1).copy()
    sdeg_pad = np.zeros(npad, dtype=np.float32)
    sdeg_pad[:N] = np.sqrt(np.maximum(deg, 1.0))
    sdeg_rows = sdeg_pad.reshape(N_CORES, 1, tiles * P).copy()

    batch_pad = np.full(npad, -1, dtype=np.int64)
    batch_pad[:N] = batch.astype(np.int64)
    g0s = np.zeros(N_CORES, dtype=np.int64)
    batchoff = np.full((N_CORES, P, tiles), float(P), dtype=np.float32)
    for k in range(N_CORES):
        b = batch_pad[k * rows_per_core:(k + 1) * rows_per_core]
        real = b >= 0
        assert real.any()
        g0s[k] = b[real].min()
        span = int(b[real].max() - g0s[k]) + 1
        assert span <= P - 1, f"graph span {span} exceeds pooling tile"
        off = np.full(rows_per_core, float(P), dtype=np.float32)
        off[real] = (b[real] - g0s[k]).astype(np.float32)
        batchoff[k] = off.reshape(tiles, P).T

    iota = np.tile(np.arange(P, dtype=np.float32), (P, ct_max))
    cnt_g = np.bincount(batch.astype(np.int64), minlength=G).astype(np.float32)

    return dict(N=N, npad=npad, H=H, rows_per_core=rows_per_core,
                tiles_per_core=tiles, c_lo=c_lo, c_hi=c_hi, sum_ca=sum_ca,
                cg_max=cg_max, ct_max=ct_max, tile_col=tile_col,
                groups=groups, idx_w=idx_w, dstoff=dstoff,
                dinv_slab=dinv_slab, sdeg_rows=sdeg_rows,
                batchoff=batchoff, g0=g0s, iota=iota, xhat=xhat, cnt_g=cnt_g)


# ---------------------------------------------------------------- device

def build_layer(pre, last_layer: bool, reps: int = 1, bf16_table: bool = True):
    """One SPMD program: grouped dma_gather + SpMM + dense matmul per dst tile.
    last_layer=False: hhat slab [rows_per_core, D] = dinv*relu(dinv*z)
    last_layer=True:  pooled [P, D] = sum over graph-offset of dinv*z
    """
    tiles = pre['tiles_per_core']
    c_lo = pre['c_lo']
    c_hi = pre['c_hi']
    sum_ca = pre['sum_ca']
    cg_max = pre['cg_max']
    ct_max = pre['ct_max']
    tile_col = pre['tile_col']
    groups = pre['groups']
    npad = pre['npad']
    H = pre['H']

    TDT = BF16 if bf16_table else F32
    nc = bacc.Bacc(get_trn_type() or "TRN2", target_bir_lowering=False, debug=False,
                   num_swdge_queues=NQ)
    table = nc.dram_tensor("table", [npad, D], TDT, kind="ExternalInput").ap()
    idx = nc.dram_tensor("idx", [P, sum_ca * (P // 16)], I16, kind="ExternalInput").ap()
    dstoff = nc.dram_tensor("dstoff", [P, sum_ca], BF16, kind="ExternalInput").ap()
    iota = nc.dram_tensor("iota", [P, ct_max * P], BF16, kind="ExternalInput").ap()
    Wt = nc.dram_tensor("W", [D, D], F32, kind="ExternalInput").ap()
    bt = nc.dram_tensor("b", [1, D], F32, kind="ExternalInput").ap()
    dinv = nc.dram_tensor("dinv", [P, tiles], F32, kind="ExternalInput").ap()
    sdeg = nc.dram_tensor("sdeg", [1, tiles * P], F32, kind="ExternalInput").ap()
    if last_layer:
        batchoff = nc.dram_tensor("batchoff", [P, tiles], BF16, kind="ExternalInput").ap()
        pooled = nc.dram_tensor("pooled", [P, D], F32, kind="ExternalOutput").ap()
    else:
        hhat = nc.dram_tensor("hhat", [tiles * P, D], TDT, kind="ExternalOutput").ap()

    with tile.TileContext(nc) as tc:
        with tc.tile_pool(name="const", bufs=1) as cp, \
             tc.tile_pool(name="gather", bufs=2) as gp, \
             tc.tile_pool(name="sel", bufs=3) as sp, \
             tc.tile_pool(name="small", bufs=3) as mp, \
             tc.tile_pool(name="hh", bufs=2) as hp, \
             tc.tile_pool(name="ps1", bufs=2, space="PSUM") as pp1, \
             tc.tile_pool(name="ps2", bufs=2, space="PSUM") as pp2:
            idx_t = cp.tile([P, sum_ca * (P // 16)], I16)
            dst_t = cp.tile([P, sum_ca], BF16)
            iota_t = cp.tile([P, ct_max * P], BF16)
            W_t = cp.tile([D, D], F32)
            b_t = cp.tile([1, D], F32)
            sdeg_t = cp.tile([1, tiles * P], F32)
            dinv_t = cp.tile([P, tiles], F32)
            nc.sync.dma_start(out=idx_t[:], in_=idx[:])
            nc.sync.dma_start(out=dst_t[:], in_=dstoff[:])
            nc.sync.dma_start(out=iota_t[:], in_=iota[:])
            nc.sync.dma_start(out=W_t[:], in_=Wt[:])
            nc.sync.dma_start(out=b_t[:], in_=bt[:])
            nc.sync.dma_start(out=sdeg_t[:], in_=sdeg[:])
            nc.sync.dma_start(out=dinv_t[:], in_=dinv[:])
            if last_layer:
                boff_t = cp.tile([P, tiles], BF16)
                nc.sync.dma_start(out=boff_t[:], in_=batchoff[:])
                pooled_sb = cp.tile([P, D], F32)

            # Per-(group, half) gathers, split into ops of <= MAX_OP_CHUNKS
            # chunks, round-robined across the SWDGE queues (a single queue
            # stalls its descriptor ring on large ops; two queues reach the
            # HBM byte roofline).
            def gather_ops(grp):
                """[(g_chunk_start, n_chunks, idx_chunk_start, table_half)]"""
                ops = []
                Lg, Hg = grp['Lg'], grp['Hg']
                co = grp['chunk_off']
                for half, base, n in ((0, 0, Lg), (1, Lg, Hg)):
                    c = 0
                    while c < n:
                        cn = min(MAX_OP_CHUNKS, n - c)
                        ops.append((base + c, cn, co + base + c, half))
                        c += cn
                return ops

            reg_cache = {}
            for grp in groups:
                for (_, cn, _, _) in gather_ops(grp):
                    if cn * P not in reg_cache:
                        reg_cache[cn * P] = nc.gpsimd.to_reg(cn * P)

            qrr = 0
            for rep in range(reps):
                if last_layer:
                    nc.vector.memset(pooled_sb[:], 0.0)
                for gi, grp in enumerate(groups):
                    ts = grp['tiles']
                    Lg, Hg = grp['Lg'], grp['Hg']
                    g = gp.tile([P, cg_max * D], TDT, tag="g")
                    for (gc0, cn, ic0, half) in gather_ops(grp):
                        src = table[0:H, :] if half == 0 else table[H:npad, :]
                        nc.gpsimd.dma_gather(
                            g[:, gc0 * D:(gc0 + cn) * D]
                                .rearrange("p (c j) -> p c j", j=D),
                            src,
                            idx_t[:, ic0 * 8:(ic0 + cn) * 8],
                            cn * P, reg_cache[cn * P], D,
                            single_packet=False, queue_num=qrr % NQ)
                        qrr += 1
                    if not last_layer:
                        hh_g = hp.tile([P, len(ts) * D], TDT, tag="hh")
                    for ti, t in enumerate(ts):
                        nlo, nhi = int(c_lo[t]), int(c_hi[t])
                        ct = nlo + nhi
                        col = int(tile_col[t])
                        S = sp.tile([P, ct_max * D], BF16, tag="s")
                        nc.vector.tensor_tensor(
                            out=S[:, :ct * D].rearrange("p (c j) -> p c j", j=D),
                            in0=dst_t[:, col:col + ct].to_broadcast([P, ct, D]),
                            in1=iota_t[:, :ct * D].rearrange("p (c j) -> p c j", j=D),
                            op=mybir.AluOpType.is_equal)
                        psumT = pp1.tile([P, D], F32, space="PSUM", tag="pT")
                        gchunks = ([grp['lo_start'][t] + c for c in range(nlo)] +
                                   [Lg + grp['hi_start'][t] + c for c in range(nhi)])
                        for ci, gc in enumerate(gchunks):
                            nc.tensor.matmul(out=psumT[:],
                                             lhsT=g[:, gc * D:(gc + 1) * D],
                                             rhs=S[:, ci * D:(ci + 1) * D],
                                             start=(ci == 0), stop=(ci == ct - 1))
                        lhs_sb = mp.tile([P, D], F32, tag="lhs")
                        nc.scalar.activation(out=lhs_sb[:], in_=psumT[:],
                                             func=mybir.ActivationFunctionType.Copy)
                        psum2 = pp2.tile([P, D], F32, space="PSUM", tag="p2")
                        nc.tensor.matmul(out=psum2[:], lhsT=lhs_sb[:], rhs=W_t[:],
                                         start=True, stop=False)
                        nc.tensor.matmul(out=psum2[:],
                                         lhsT=sdeg_t[:, t * P:(t + 1) * P],
                                         rhs=b_t[:], start=False, stop=True)
                        if last_layer:
                            out_sb = mp.tile([P, D], F32, tag="out")
                            nc.scalar.activation(out=out_sb[:], in_=psum2[:],
                                                 func=mybir.ActivationFunctionType.Copy,
                                                 scale=dinv_t[:, t:t + 1])
                            Pt = sp.tile([P, D], F32, tag="pool_sel")
                            nc.vector.tensor_tensor(
                                out=Pt[:],
                                in0=boff_t[:, t:t + 1].to_broadcast([P, D]),
                                in1=iota_t[:, :D],
                                op=mybir.AluOpType.is_equal)
                            pool_ps = pp2.tile([P, D], F32, space="PSUM", tag="pool_ps")
                            nc.tensor.matmul(out=pool_ps[:], lhsT=Pt[:], rhs=out_sb[:],
                                             start=True, stop=True)
                            nc.vector.tensor_add(out=pooled_sb[:], in0=pooled_sb[:],
                                                 in1=pool_ps[:])
                        else:
                            out_sb = mp.tile([P, D], F32, tag="out")
                            nc.scalar.activation(out=out_sb[:], in_=psum2[:],
                                                 func=mybir.ActivationFunctionType.Relu,
                                                 scale=dinv_t[:, t:t + 1])
                            nc.vector.tensor_scalar_mul(
                                out=hh_g[:, ti * D:(ti + 1) * D], in0=out_sb[:],
                                scalar1=dinv_t[:, t:t + 1])
                    if not last_layer:
                        t0 = ts[0]
                        nc.sync.dma_start(
                            out=hhat[t0 * P:(t0 + len(ts)) * P, :]
                                .rearrange("(c p) j -> p c j", p=P),
                            in_=hh_g[:, :len(ts) * D].rearrange("p (c j) -> p c j", j=D))
                if last_layer:
                    nc.sync.dma_start(out=pooled[:], in_=pooled_sb[:])
    nc.compile()
    return nc


def _in_maps(pre, table_np, W, b, last_layer):
    maps = []
    for k in range(N_CORES):
        m = dict(table=table_np,
                 idx=pre['idx_w'][k],
                 dstoff=pre['dstoff'][k].astype(ml_dtypes.bfloat16),
                 iota=pre['iota'].astype(ml_dtypes.bfloat16),
                 W=np.ascontiguousarray(W, dtype=np.float32),
                 b=np.ascontiguousarray(b, dtype=np.float32).reshape(1, D),
                 dinv=pre['dinv_slab'][k],
                 sdeg=pre['sdeg_rows'][k])
        if last_layer:
            m['batchoff'] = pre['batchoff'][k].astype(ml_dtypes.bfloat16)
        maps.append(m)
    return maps


def kernel(x, edge_index, batch, W1, b1, W2, b2):
    x = np.asarray(x); edge_index = np.asarray(edge_index)
    batch = np.asarray(batch)
    W1 = np.asarray(W1); b1 = np.asarray(b1)
    W2 = np.asarray(W2); b2 = np.asarray(b2)

    pre = preprocess(x, edge_index, batch)
    core_ids = list(range(N_CORES))

    tdt = ml_dtypes.bfloat16 if USE_BF16 else np.float32
    table1 = pre['xhat'].astype(tdt)
    nc1 = build_layer(pre, last_layer=False, bf16_table=USE_BF16)
    res1 = run_bass_kernel_spmd(nc1, _in_maps(pre, table1, W1, b1, False),
                                core_ids).results

    h1hat = np.zeros((pre['npad'], D), dtype=tdt)
    rpc = pre['rows_per_core']
    for k in range(N_CORES):
        h1hat[k * rpc:(k + 1) * rpc] = res1[k]['hhat']

    nc2 = build_layer(pre, last_layer=True, bf16_table=USE_BF16)
    res2 = run_bass_kernel_spmd(nc2, _in_maps(pre, h1hat, W2, b2, True),
                                core_ids).results

    pooled = np.zeros((G, D), dtype=np.float32)
    for k in range(N_CORES):
        part = res2[k]['pooled']
        g0 = int(pre['g0'][k])
        span = min(P, G - g0)
        pooled[g0:g0 + span] += part[:span]
    return pooled / np.maximum(pre['cnt_g'], 1.0)[:, None]


# revision 12
# speedup vs baseline: 3.5539x; 1.3200x over previous
"""Trainium2 Bass kernel for a 2-layer GCN encoder (GCNConv x2 + global mean pool).

Math: with A' = A + I and deg = indegree(A') (symmetric-norm GCN),
    gcn(h, W, b) = D^-1/2 A' D^-1/2 (h) W + b
factorized as  out = dinv * (A' @ (dinv * h)) @ W + b   (dinv = deg^-1/2)
so the SpMM is pure 0/1 structure; per-edge norms become per-node row scales.

Sharding: dst-node ranges across 8 cores (6272 rows / 49 tiles each). The
per-edge source-row gather uses gpsimd `dma_gather` (one SWDGE op covers
thousands of rows at ~0.34ns/descriptor) instead of per-128-row indirect
DMAs (994ns fixed each) — that fixed overhead was the previous bottleneck.
dma_gather indices are int16, so the (replicated) HBM feature table is
addressed as two 25088-row halves; each dst tile's edge list (self-loops
included as ordinary indices) is split by source half. Tiles are processed
in groups of 7: two dma_gathers per group (low/high half) land all chunks
in SBUF in the [slot%128 -> partition, slot//128 -> chunk] layout the
scatter expects. Per tile, a bf16 0/1 scatter matrix (dst-offset vs iota
compare on VectorE) feeds TensorE one-hot matmuls accumulating psumT
[feat, dstoff] in PSUM, then the dense W matmul + rank-1 bias matmul +
fused relu/dinv scaling as before. Two SPMD launches (layer 1 -> host
allgather of the 1.6MB/core slabs -> layer 2 + graph pooling).
"""
import math
import numpy as np
import ml_dtypes

from concourse import bass, mybir, tile, bacc
from concourse.bass_utils import run_bass_kernel_spmd
from concourse._compat import get_trn_type

N_CORES = 8
P = 128          # partitions / tile rows
D = 128          # feature dim
G = 512          # number of graphs (fixed by the problem)
GROUP = 7        # dst tiles per gather group (49 = 7*7)
NQ = 4           # SWDGE descriptor-generation queues
MAX_OP_CHUNKS = 35   # max 128-row chunks per dma_gather op (ring-safe)
F32 = mybir.dt.float32
BF16 = mybir.dt.bfloat16
I16 = mybir.dt.int16

USE_BF16 = True     # bf16 gather table (half the random-gather bytes; rel err ~1e-4)


# ---------------------------------------------------------------- host prep

def _pad_chunks(idx, off, nchunks):
    """Pad (idx, off) to nchunks*P slots: idx pads with 0 (valid row,
    masked by off=P in the scatter matrix)."""
    n = nchunks * P
    ip = np.zeros(n, dtype=np.int16)
    ip[:len(idx)] = idx
    op = np.full(n, float(P), dtype=np.float32)
    op[:len(off)] = off
    return ip, op


def preprocess(x, edge_index, batch):
    N = x.shape[0]
    rows_per_core = int(math.ceil(N / (N_CORES * P))) * P
    npad = rows_per_core * N_CORES
    tiles = rows_per_core // P
    n_tiles_g = N_CORES * tiles
    H = npad // 2
    assert H % P == 0 and rows_per_core % P == 0
    ngroups = (tiles + GROUP - 1) // GROUP

    src = edge_index[0].astype(np.int64)
    dst = edge_index[1].astype(np.int64)
    deg = (np.bincount(dst, minlength=N) + 1).astype(np.float32)
    dinv = 1.0 / np.sqrt(np.maximum(deg, 1.0))

    xhat = np.zeros((npad, D), dtype=np.float32)
    xhat[:N] = x.astype(np.float32) * dinv[:, None]

    order = np.argsort(dst)
    src_s = src[order]
    dst_s = dst[order]
    bounds = np.searchsorted(dst_s, np.arange(0, npad + 1, P))

    # Per (core, tile): edge + self-loop source lists split by table half.
    lo_lists = [[None] * tiles for _ in range(N_CORES)]
    hi_lists = [[None] * tiles for _ in range(N_CORES)]
    selfid = np.arange(P, dtype=np.int64)
    for k in range(N_CORES):
        for t in range(tiles):
            gt = k * tiles + t
            s, e = int(bounds[gt]), int(bounds[gt + 1])
            asrc = np.concatenate([src_s[s:e], gt * P + selfid])
            aoff = np.concatenate([(dst_s[s:e] - gt * P).astype(np.float32),
                                   selfid.astype(np.float32)])
            m = asrc < H
            lo_lists[k][t] = (asrc[m].astype(np.int16), aoff[m])
            hi_lists[k][t] = ((asrc[~m] - H).astype(np.int16), aoff[~m])

    c_lo = np.array([max(-(-len(lo_lists[k][t][0]) // P) for k in range(N_CORES))
                     for t in range(tiles)], dtype=np.int64)
    c_hi = np.array([max(-(-len(hi_lists[k][t][0]) // P) for k in range(N_CORES))
                     for t in range(tiles)], dtype=np.int64)

    # Static group metadata (identical across cores -> SPMD-uniform program).
    groups = []          # per group: dict with tile list + chunk bookkeeping
    sum_ca = int((c_lo + c_hi).sum())
    cg_max = 0
    ct_max = 0
    tile_col = np.zeros(tiles + 1, dtype=np.int64)   # dstoff col offset, tile-major
    for t in range(tiles):
        tile_col[t + 1] = tile_col[t] + c_lo[t] + c_hi[t]
        ct_max = max(ct_max, int(c_lo[t] + c_hi[t]))
    gather_chunk_off = 0     # running chunk offset in gather order
    for g0 in range(0, tiles, GROUP):
        ts = list(range(g0, min(g0 + GROUP, tiles)))
        Lg = int(sum(c_lo[t] for t in ts))
        Hg = int(sum(c_hi[t] for t in ts))
        lo_start = {}
        hi_start = {}
        acc = 0
        for t in ts:
            lo_start[t] = acc
            acc += int(c_lo[t])
        acc = 0
        for t in ts:
            hi_start[t] = acc
            acc += int(c_hi[t])
        groups.append(dict(tiles=ts, Lg=Lg, Hg=Hg, lo_start=lo_start,
                           hi_start=hi_start, chunk_off=gather_chunk_off))
        gather_chunk_off += Lg + Hg
        cg_max = max(cg_max, Lg + Hg)
    assert gather_chunk_off == sum_ca

    # Per-core flat streams.
    #  - idx stream in GATHER order: per group, lo chunks (tile-major) then hi.
    #  - dstoff in TILE-major order: per tile, lo chunks then hi chunks.
    idx_w = np.zeros((N_CORES, P, sum_ca * (P // 16)), dtype=np.int16)
    dstoff = np.full((N_CORES, P, sum_ca), float(P), dtype=np.float32)
    for k in range(N_CORES):
        stream = []
        for g in groups:
            for t in g['tiles']:
                ip, _ = _pad_chunks(*lo_lists[k][t], int(c_lo[t]))
                stream.append(ip)
            for t in g['tiles']:
                ip, _ = _pad_chunks(*hi_lists[k][t], int(c_hi[t]))
                stream.append(ip)
        stream = np.concatenate(stream)
        assert stream.shape[0] == sum_ca * P
        idx_w[k] = np.tile(stream.reshape(-1, 16).T, (8, 1))
        for t in range(tiles):
            col = int(tile_col[t])
            _, op_ = _pad_chunks(*lo_lists[k][t], int(c_lo[t]))
            if c_lo[t]:
                dstoff[k, :, col:col + int(c_lo[t])] = op_.reshape(-1, P).T
            _, op_ = _pad_chunks(*hi_lists[k][t], int(c_hi[t]))
            if c_hi[t]:
                dstoff[k, :, col + int(c_lo[t]):col + int(c_lo[t] + c_hi[t])] = \
                    op_.reshape(-1, P).T

    dinv_pad = np.zeros(npad, dtype=np.float32)
    dinv_pad[:N] = dinv
    dinv_slab = dinv_pad.reshape(N_CORES, tiles, P).transpose(0, 2, 1).copy()
    sdeg_pad = np.zeros(npad, dtype=np.float32)
    sdeg_pad[:N] = np.sqrt(np.maximum(deg, 1.0))
    sdeg_rows = sdeg_pad.reshape(N_CORES, 1, tiles * P).copy()

    batch_pad = np.full(npad, -1, dtype=np.int64)
    batch_pad[:N] = batch.astype(np.int64)
    g0s = np.zeros(N_CORES, dtype=np.int64)
    batchoff = np.full((N_CORES, P, tiles), float(P), dtype=np.float32)
    for k in range(N_CORES):
        b = batch_pad[k * rows_per_core:(k + 1) * rows_per_core]
        real = b >= 0
        assert real.any()
        g0s[k] = b[real].min()
        span = int(b[real].max() - g0s[k]) + 1
        assert span <= P - 1, f"graph span {span} exceeds pooling tile"
        off = np.full(rows_per_core, float(P), dtype=np.float32)
        off[real] = (b[real] - g0s[k]).astype(np.float32)
        batchoff[k] = off.reshape(tiles, P).T

    iota = np.tile(np.arange(P, dtype=np.float32), (P, ct_max))
    cnt_g = np.bincount(batch.astype(np.int64), minlength=G).astype(np.float32)

    return dict(N=N, npad=npad, H=H, rows_per_core=rows_per_core,
                tiles_per_core=tiles, c_lo=c_lo, c_hi=c_hi, sum_ca=sum_ca,
                cg_max=cg_max, ct_max=ct_max, tile_col=tile_col,
                groups=groups, idx_w=idx_w, dstoff=dstoff,
                dinv_slab=dinv_slab, sdeg_rows=sdeg_rows,
                batchoff=batchoff, g0=g0s, iota=iota, xhat=xhat, cnt_g=cnt_g)


# ---------------------------------------------------------------- device

def build_layer(pre, last_layer: bool, reps: int = 1, bf16_table: bool = True):
    """One SPMD program: grouped dma_gather + SpMM + dense matmul per dst tile.
    last_layer=False: hhat slab [rows_per_core, D] = dinv*relu(dinv*z)
    last_layer=True:  pooled [P, D] = sum over graph-offset of dinv*z
    """
    tiles = pre['tiles_per_core']
    c_lo = pre['c_lo']
    c_hi = pre['c_hi']
    sum_ca = pre['sum_ca']
    cg_max = pre['cg_max']
    ct_max = pre['ct_max']
    tile_col = pre['tile_col']
    groups = pre['groups']
    npad = pre['npad']
    H = pre['H']

    TDT = BF16 if bf16_table else F32
    nc = bacc.Bacc(get_trn_type() or "TRN2", target_bir_lowering=False, debug=False,
                   num_swdge_queues=NQ)
    table = nc.dram_tensor("table", [npad, D], TDT, kind="ExternalInput").ap()
    idx = nc.dram_tensor("idx", [P, sum_ca * (P // 16)], I16, kind="ExternalInput").ap()
    dstoff = nc.dram_tensor("dstoff", [P, sum_ca], BF16, kind="ExternalInput").ap()
    iota = nc.dram_tensor("iota", [P, ct_max * P], BF16, kind="ExternalInput").ap()
    Wt = nc.dram_tensor("W", [D, D], F32, kind="ExternalInput").ap()
    bt = nc.dram_tensor("b", [1, D], F32, kind="ExternalInput").ap()
    dinv = nc.dram_tensor("dinv", [P, tiles], F32, kind="ExternalInput").ap()
    sdeg = nc.dram_tensor("sdeg", [1, tiles * P], F32, kind="ExternalInput").ap()
    if last_layer:
        batchoff = nc.dram_tensor("batchoff", [P, tiles], BF16, kind="ExternalInput").ap()
        pooled = nc.dram_tensor("pooled", [P, D], F32, kind="ExternalOutput").ap()
    else:
        hhat = nc.dram_tensor("hhat", [tiles * P, D], TDT, kind="ExternalOutput").ap()

    with tile.TileContext(nc) as tc:
        with tc.tile_pool(name="const", bufs=1) as cp, \
             tc.tile_pool(name="gather", bufs=2) as gp, \
             tc.tile_pool(name="sel", bufs=3) as sp, \
             tc.tile_pool(name="small", bufs=3) as mp, \
             tc.tile_pool(name="hh", bufs=2) as hp, \
             tc.tile_pool(name="ps1", bufs=2, space="PSUM") as pp1, \
             tc.tile_pool(name="ps2", bufs=2, space="PSUM") as pp2:
            idx_t = cp.tile([P, sum_ca * (P // 16)], I16)
            dst_t = cp.tile([P, sum_ca], BF16)
            iota_t = cp.tile([P, ct_max * P], BF16)
            W_t = cp.tile([D, D], F32)
            b_t = cp.tile([1, D], F32)
            sdeg_t = cp.tile([1, tiles * P], F32)
            dinv_t = cp.tile([P, tiles], F32)
            nc.sync.dma_start(out=idx_t[:], in_=idx[:])
            nc.sync.dma_start(out=dst_t[:], in_=dstoff[:])
            nc.sync.dma_start(out=iota_t[:], in_=iota[:])
            nc.sync.dma_start(out=W_t[:], in_=Wt[:])
            nc.sync.dma_start(out=b_t[:], in_=bt[:])
            nc.sync.dma_start(out=sdeg_t[:], in_=sdeg[:])
            nc.sync.dma_start(out=dinv_t[:], in_=dinv[:])
            if last_layer:
                boff_t = cp.tile([P, tiles], BF16)
                nc.sync.dma_start(out=boff_t[:], in_=batchoff[:])
                pooled_sb = cp.tile([P, D], F32)

            # Per-(group, half) gathers, split into ops of <= MAX_OP_CHUNKS
            # chunks, round-robined across the SWDGE queues (a single queue
            # stalls its descriptor ring on large ops; two queues reach the
            # HBM byte roofline).
            def gather_ops(grp):
                """[(g_chunk_start, n_chunks, idx_chunk_start, table_half)]"""
                ops = []
                Lg, Hg = grp['Lg'], grp['Hg']
                co = grp['chunk_off']
                for half, base, n in ((0, 0, Lg), (1, Lg, Hg)):
                    c = 0
                    while c < n:
                        cn = min(MAX_OP_CHUNKS, n - c)
                        ops.append((base + c, cn, co + base + c, half))
                        c += cn
                return ops

            reg_cache = {}
            for grp in groups:
                for (_, cn, _, _) in gather_ops(grp):
                    if cn * P not in reg_cache:
                        reg_cache[cn * P] = nc.gpsimd.to_reg(cn * P)

            qrr = 0
            for rep in range(reps):
                if last_layer:
                    nc.vector.memset(pooled_sb[:], 0.0)
                for gi, grp in enumerate(groups):
                    ts = grp['tiles']
                    Lg, Hg = grp['Lg'], grp['Hg']
                    g = gp.tile([P, cg_max * D], TDT, tag="g")
                    for (gc0, cn, ic0, half) in gather_ops(grp):
                        src = table[0:H, :] if half == 0 else table[H:npad, :]
                        nc.gpsimd.dma_gather(
                            g[:, gc0 * D:(gc0 + cn) * D]
                                .rearrange("p (c j) -> p c j", j=D),
                            src,
                            idx_t[:, ic0 * 8:(ic0 + cn) * 8],
                            cn * P, reg_cache[cn * P], D,
                            single_packet=False, queue_num=qrr % NQ)
                        qrr += 1
                    if not last_layer:
                        hh_g = hp.tile([P, len(ts) * D], TDT, tag="hh")
                    for ti, t in enumerate(ts):
                        nlo, nhi = int(c_lo[t]), int(c_hi[t])
                        ct = nlo + nhi
                        col = int(tile_col[t])
                        S = sp.tile([P, ct_max * D], BF16, tag="s")
                        nc.vector.tensor_tensor(
                            out=S[:, :ct * D].rearrange("p (c j) -> p c j", j=D),
                            in0=dst_t[:, col:col + ct].to_broadcast([P, ct, D]),
                            in1=iota_t[:, :ct * D].rearrange("p (c j) -> p c j", j=D),
                            op=mybir.AluOpType.is_equal)
                        psumT = pp1.tile([P, D], F32, space="PSUM", tag="pT")
                        gchunks = ([grp['lo_start'][t] + c for c in range(nlo)] +
                                   [Lg + grp['hi_start'][t] + c for c in range(nhi)])
                        for ci, gc in enumerate(gchunks):
                            nc.tensor.matmul(out=psumT[:],
                                             lhsT=g[:, gc * D:(gc + 1) * D],
                                             rhs=S[:, ci * D:(ci + 1) * D],
                                             start=(ci == 0), stop=(ci == ct - 1))
                        lhs_sb = mp.tile([P, D], F32, tag="lhs")
                        nc.scalar.activation(out=lhs_sb[:], in_=psumT[:],
                                             func=mybir.ActivationFunctionType.Copy)
                        psum2 = pp2.tile([P, D], F32, space="PSUM", tag="p2")
                        nc.tensor.matmul(out=psum2[:], lhsT=lhs_sb[:], rhs=W_t[:],
                                         start=True, stop=False)
                        nc.tensor.matmul(out=psum2[:],
                                         lhsT=sdeg_t[:, t * P:(t + 1) * P],
                                         rhs=b_t[:], start=False, stop=True)
                        if last_layer:
                            out_sb = mp.tile([P, D], F32, tag="out")
                            nc.scalar.activation(out=out_sb[:], in_=psum2[:],
                                                 func=mybir.ActivationFunctionType.Copy,
                                                 scale=dinv_t[:, t:t + 1])
                            Pt = sp.tile([P, D], F32, tag="pool_sel")
                            nc.vector.tensor_tensor(
                                out=Pt[:],
                                in0=boff_t[:, t:t + 1].to_broadcast([P, D]),
                                in1=iota_t[:, :D],
                                op=mybir.AluOpType.is_equal)
                            pool_ps = pp2.tile([P, D], F32, space="PSUM", tag="pool_ps")
                            nc.tensor.matmul(out=pool_ps[:], lhsT=Pt[:], rhs=out_sb[:],
                                             start=True, stop=True)
                            nc.vector.tensor_add(out=pooled_sb[:], in0=pooled_sb[:],
                                                 in1=pool_ps[:])
                        else:
                            out_sb = mp.tile([P, D], F32, tag="out")
                            nc.scalar.activation(out=out_sb[:], in_=psum2[:],
                                                 func=mybir.ActivationFunctionType.Relu,
                                                 scale=dinv_t[:, t:t + 1])
                            nc.vector.tensor_scalar_mul(
                                out=hh_g[:, ti * D:(ti + 1) * D], in0=out_sb[:],
                                scalar1=dinv_t[:, t:t + 1])
                    if not last_layer:
                        t0 = ts[0]
                        nc.sync.dma_start(
                            out=hhat[t0 * P:(t0 + len(ts)) * P, :]
                                .rearrange("(c p) j -> p c j", p=P),
                            in_=hh_g[:, :len(ts) * D].rearrange("p (c j) -> p c j", j=D))
                if last_layer:
                    nc.sync.dma_start(out=pooled[:], in_=pooled_sb[:])
    nc.compile()
    return nc


def _in_maps(pre, table_np, W, b, last_layer):
    maps = []
    for k in range(N_CORES):
        m = dict(table=table_np,
                 idx=pre['idx_w'][k],
                 dstoff=pre['dstoff'][k].astype(ml_dtypes.bfloat16),
                 iota=pre['iota'].astype(ml_dtypes.bfloat16),
                 W=np.ascontiguousarray(W, dtype=np.float32),
                 b=np.ascontiguousarray(b, dtype=np.float32).reshape(1, D),
                 dinv=pre['dinv_slab'][k],
                 sdeg=pre['sdeg_rows'][k])
        if last_layer:
            m['batchoff'] = pre['batchoff'][k].astype(ml_dtypes.bfloat16)
        maps.append(m)
    return maps


def kernel(x, edge_index, batch, W1, b1, W2, b2):
    x = np.asarray(x); edge_index = np.asarray(edge_index)
    batch = np.asarray(batch)
    W1 = np.asarray(W1); b1 = np.asarray(b1)
    W2 = np.asarray(W2); b2 = np.asarray(b2)

    pre = preprocess(x, edge_index, batch)
    core_ids = list(range(N_CORES))

    tdt = ml_dtypes.bfloat16 if USE_BF16 else np.float32
    table1 = pre['xhat'].astype(tdt)
    nc1 = build_layer(pre, last_layer=False, bf16_table=USE_BF16)
    res1 = run_bass_kernel_spmd(nc1, _in_maps(pre, table1, W1, b1, False),
                                core_ids).results

    h1hat = np.zeros((pre['npad'], D), dtype=tdt)
    rpc = pre['rows_per_core']
    for k in range(N_CORES):
        h1hat[k * rpc:(k + 1) * rpc] = res1[k]['hhat']

    nc2 = build_layer(pre, last_layer=True, bf16_table=USE_BF16)
    res2 = run_bass_kernel_spmd(nc2, _in_maps(pre, h1hat, W2, b2, True),
                                core_ids).results

    pooled = np.zeros((G, D), dtype=np.float32)
    for k in range(N_CORES):
        part = res2[k]['pooled']
        g0 = int(pre['g0'][k])
        span = min(P, G - g0)
        pooled[g0:g0 + span] += part[:span]
    return pooled / np.maximum(pre['cnt_g'], 1.0)[:, None]


# revision 17
# speedup vs baseline: 6.0995x; 1.7163x over previous
"""Trainium2 Bass kernel for a 2-layer GCN encoder (GCNConv x2 + global mean pool).

Math: with A' = A + I and deg = indegree(A') (symmetric-norm GCN),
    gcn(h, W, b) = D^-1/2 A' D^-1/2 (h) W + b
factorized as  out = dinv * (A' @ (dinv * h)) @ W + b   (dinv = deg^-1/2)
so the SpMM is pure 0/1 structure; per-edge norms become per-node row scales.

Sharding: dst-node ranges across 8 cores (6272 rows / 49 tiles each). The
per-edge source-row gather uses gpsimd `dma_gather` (one SWDGE op covers
thousands of rows at ~0.34ns/descriptor) instead of per-128-row indirect
DMAs (994ns fixed each) — that fixed overhead was the previous bottleneck.
dma_gather indices are int16, so the (replicated) HBM feature table is
addressed as two 25088-row halves; each dst tile's edge list (self-loops
included as ordinary indices) is split by source half. Tiles are processed
in groups of 7: two dma_gathers per group (low/high half) land all chunks
in SBUF in the [slot%128 -> partition, slot//128 -> chunk] layout the
scatter expects. Per tile, a bf16 0/1 scatter matrix (dst-offset vs iota
compare on VectorE) feeds TensorE one-hot matmuls accumulating psumT
[feat, dstoff] in PSUM, then the dense W matmul + rank-1 bias matmul +
fused relu/dinv scaling as before. Two SPMD launches (layer 1 -> host
allgather of the 1.6MB/core slabs -> layer 2 + graph pooling).
"""
import math
import numpy as np
import ml_dtypes

from concourse import bass, mybir, tile, bacc
from concourse.bass_utils import run_bass_kernel_spmd
from concourse._compat import get_trn_type

N_CORES = 8
P = 128          # partitions / tile rows
D = 128          # feature dim
G = 512          # number of graphs (fixed by the problem)
GROUP = 7        # dst tiles per gather group (49 = 7*7)
NQ = 4           # SWDGE descriptor-generation queues
MAX_OP_CHUNKS = 35   # max 128-row chunks per dma_gather op (ring-safe)
F32 = mybir.dt.float32
BF16 = mybir.dt.bfloat16
I16 = mybir.dt.int16

USE_BF16 = True     # bf16 gather table (half the random-gather bytes; rel err ~1e-4)


# ---------------------------------------------------------------- host prep

def _pad_chunks(idx, off, nchunks):
    """Pad (idx, off) to nchunks*P slots: idx pads with 0 (valid row,
    masked by off=P in the scatter matrix)."""
    n = nchunks * P
    ip = np.zeros(n, dtype=np.int16)
    ip[:len(idx)] = idx
    op = np.full(n, float(P), dtype=np.float32)
    op[:len(off)] = off
    return ip, op


def preprocess(x, edge_index, batch):
    N = x.shape[0]
    rows_per_core = int(math.ceil(N / (N_CORES * P))) * P
    npad = rows_per_core * N_CORES
    tiles = rows_per_core // P
    n_tiles_g = N_CORES * tiles
    H = npad // 2
    assert H % P == 0 and rows_per_core % P == 0
    ngroups = (tiles + GROUP - 1) // GROUP

    src = edge_index[0].astype(np.int64)
    dst = edge_index[1].astype(np.int64)
    deg = (np.bincount(dst, minlength=N) + 1).astype(np.float32)
    dinv = 1.0 / np.sqrt(np.maximum(deg, 1.0))

    xhat = np.zeros((npad, D), dtype=np.float32)
    xhat[:N] = x.astype(np.float32) * dinv[:, None]

    order = np.argsort(dst)
    src_s = src[order]
    dst_s = dst[order]
    bounds = np.searchsorted(dst_s, np.arange(0, npad + 1, P))

    # Per (core, tile): edge + self-loop source lists split by table half.
    lo_lists = [[None] * tiles for _ in range(N_CORES)]
    hi_lists = [[None] * tiles for _ in range(N_CORES)]
    selfid = np.arange(P, dtype=np.int64)
    for k in range(N_CORES):
        for t in range(tiles):
            gt = k * tiles + t
            s, e = int(bounds[gt]), int(bounds[gt + 1])
            asrc = np.concatenate([src_s[s:e], gt * P + selfid])
            aoff = np.concatenate([(dst_s[s:e] - gt * P).astype(np.float32),
                                   selfid.astype(np.float32)])
            m = asrc < H
            lo_lists[k][t] = (asrc[m].astype(np.int16), aoff[m])
            hi_lists[k][t] = ((asrc[~m] - H).astype(np.int16), aoff[~m])

    c_lo = np.array([max(-(-len(lo_lists[k][t][0]) // P) for k in range(N_CORES))
                     for t in range(tiles)], dtype=np.int64)
    c_hi = np.array([max(-(-len(hi_lists[k][t][0]) // P) for k in range(N_CORES))
                     for t in range(tiles)], dtype=np.int64)

    # Static group metadata (identical across cores -> SPMD-uniform program).
    groups = []          # per group: dict with tile list + chunk bookkeeping
    sum_ca = int((c_lo + c_hi).sum())
    cg_max = 0
    ct_max = 0
    tile_col = np.zeros(tiles + 1, dtype=np.int64)   # dstoff col offset, tile-major
    for t in range(tiles):
        tile_col[t + 1] = tile_col[t] + c_lo[t] + c_hi[t]
        ct_max = max(ct_max, int(c_lo[t] + c_hi[t]))
    gather_chunk_off = 0     # running chunk offset in gather order
    for g0 in range(0, tiles, GROUP):
        ts = list(range(g0, min(g0 + GROUP, tiles)))
        Lg = int(sum(c_lo[t] for t in ts))
        Hg = int(sum(c_hi[t] for t in ts))
        lo_start = {}
        hi_start = {}
        acc = 0
        for t in ts:
            lo_start[t] = acc
            acc += int(c_lo[t])
        acc = 0
        for t in ts:
            hi_start[t] = acc
            acc += int(c_hi[t])
        groups.append(dict(tiles=ts, Lg=Lg, Hg=Hg, lo_start=lo_start,
                           hi_start=hi_start, chunk_off=gather_chunk_off))
        gather_chunk_off += Lg + Hg
        cg_max = max(cg_max, Lg + Hg)
    assert gather_chunk_off == sum_ca

    # Per-core flat streams.
    #  - idx stream in GATHER order: per group, lo chunks (tile-major) then hi.
    #  - dstoff in TILE-major order: per tile, lo chunks then hi chunks.
    idx_w = np.zeros((N_CORES, P, sum_ca * (P // 16)), dtype=np.int16)
    dstoff = np.full((N_CORES, P, sum_ca), float(P), dtype=np.float32)
    for k in range(N_CORES):
        stream = []
        for g in groups:
            for t in g['tiles']:
                ip, _ = _pad_chunks(*lo_lists[k][t], int(c_lo[t]))
                stream.append(ip)
            for t in g['tiles']:
                ip, _ = _pad_chunks(*hi_lists[k][t], int(c_hi[t]))
                stream.append(ip)
        stream = np.concatenate(stream)
        assert stream.shape[0] == sum_ca * P
        idx_w[k] = np.tile(stream.reshape(-1, 16).T, (8, 1))
        for t in range(tiles):
            col = int(tile_col[t])
            _, op_ = _pad_chunks(*lo_lists[k][t], int(c_lo[t]))
            if c_lo[t]:
                dstoff[k, :, col:col + int(c_lo[t])] = op_.reshape(-1, P).T
            _, op_ = _pad_chunks(*hi_lists[k][t], int(c_hi[t]))
            if c_hi[t]:
                dstoff[k, :, col + int(c_lo[t]):col + int(c_lo[t] + c_hi[t])] = \
                    op_.reshape(-1, P).T

    dinv_pad = np.zeros(npad, dtype=np.float32)
    dinv_pad[:N] = dinv
    dinv_slab = dinv_pad.reshape(N_CORES, tiles, P).transpose(0, 2, 1).copy()
    sdeg_pad = np.zeros(npad, dtype=np.float32)
    sdeg_pad[:N] = np.sqrt(np.maximum(deg, 1.0))
    sdeg_rows = sdeg_pad.reshape(N_CORES, 1, tiles * P).copy()

    batch_pad = np.full(npad, -1, dtype=np.int64)
    batch_pad[:N] = batch.astype(np.int64)
    g0s = np.zeros(N_CORES, dtype=np.int64)
    batchoff = np.full((N_CORES, P, tiles), float(P), dtype=np.float32)
    for k in range(N_CORES):
        b = batch_pad[k * rows_per_core:(k + 1) * rows_per_core]
        real = b >= 0
        assert real.any()
        g0s[k] = b[real].min()
        span = int(b[real].max() - g0s[k]) + 1
        assert span <= P - 1, f"graph span {span} exceeds pooling tile"
        off = np.full(rows_per_core, float(P), dtype=np.float32)
        off[real] = (b[real] - g0s[k]).astype(np.float32)
        batchoff[k] = off.reshape(tiles, P).T

    iota = np.tile(np.arange(P, dtype=np.float32), (P, ct_max))
    cnt_g = np.bincount(batch.astype(np.int64), minlength=G).astype(np.float32)

    return dict(N=N, npad=npad, H=H, rows_per_core=rows_per_core,
                tiles_per_core=tiles, c_lo=c_lo, c_hi=c_hi, sum_ca=sum_ca,
                cg_max=cg_max, ct_max=ct_max, tile_col=tile_col,
                groups=groups, idx_w=idx_w, dstoff=dstoff,
                dinv_slab=dinv_slab, sdeg_rows=sdeg_rows,
                batchoff=batchoff, g0=g0s, iota=iota, xhat=xhat, cnt_g=cnt_g)


# ---------------------------------------------------------------- device

def build_layer(pre, last_layer: bool, reps: int = 1, bf16_table: bool = True):
    """One SPMD program: grouped dma_gather + SpMM + dense matmul per dst tile.
    last_layer=False: hhat slab [rows_per_core, D] = dinv*relu(dinv*z)
    last_layer=True:  pooled [P, D] = sum over graph-offset of dinv*z
    """
    tiles = pre['tiles_per_core']
    c_lo = pre['c_lo']
    c_hi = pre['c_hi']
    sum_ca = pre['sum_ca']
    cg_max = pre['cg_max']
    ct_max = pre['ct_max']
    tile_col = pre['tile_col']
    groups = pre['groups']
    npad = pre['npad']
    H = pre['H']

    TDT = BF16 if bf16_table else F32
    nc = bacc.Bacc(get_trn_type() or "TRN2", target_bir_lowering=False, debug=False,
                   num_swdge_queues=NQ)
    table = nc.dram_tensor("table", [npad, D], TDT, kind="ExternalInput").ap()
    idx = nc.dram_tensor("idx", [P, sum_ca * (P // 16)], I16, kind="ExternalInput").ap()
    dstoff = nc.dram_tensor("dstoff", [P, sum_ca], BF16, kind="ExternalInput").ap()
    iota = nc.dram_tensor("iota", [P, ct_max * P], BF16, kind="ExternalInput").ap()
    Wt = nc.dram_tensor("W", [D, D], F32, kind="ExternalInput").ap()
    bt = nc.dram_tensor("b", [1, D], F32, kind="ExternalInput").ap()
    dinv = nc.dram_tensor("dinv", [P, tiles], F32, kind="ExternalInput").ap()
    sdeg = nc.dram_tensor("sdeg", [1, tiles * P], F32, kind="ExternalInput").ap()
    if last_layer:
        batchoff = nc.dram_tensor("batchoff", [P, tiles], BF16, kind="ExternalInput").ap()
        pooled = nc.dram_tensor("pooled", [P, D], F32, kind="ExternalOutput").ap()
    else:
        hhat = nc.dram_tensor("hhat", [tiles * P, D], TDT, kind="ExternalOutput").ap()

    with tile.TileContext(nc) as tc:
        with tc.tile_pool(name="const", bufs=1) as cp, \
             tc.tile_pool(name="gather", bufs=2) as gp, \
             tc.tile_pool(name="sel", bufs=3) as sp, \
             tc.tile_pool(name="small", bufs=3) as mp, \
             tc.tile_pool(name="hh", bufs=2) as hp, \
             tc.tile_pool(name="ps1", bufs=2, space="PSUM") as pp1, \
             tc.tile_pool(name="ps2", bufs=2, space="PSUM") as pp2, \
             tc.tile_pool(name="ps3", bufs=2, space="PSUM") as pp3:
            idx_t = cp.tile([P, sum_ca * (P // 16)], I16)
            dst_t = cp.tile([P, sum_ca], BF16)
            iota_t = cp.tile([P, ct_max * P], BF16)
            W_t = cp.tile([D, D], F32)
            b_t = cp.tile([1, D], F32)
            sdeg_t = cp.tile([1, tiles * P], F32)
            dinv_t = cp.tile([P, tiles], F32)
            nc.sync.dma_start(out=idx_t[:], in_=idx[:])
            nc.sync.dma_start(out=dst_t[:], in_=dstoff[:])
            nc.sync.dma_start(out=iota_t[:], in_=iota[:])
            nc.sync.dma_start(out=W_t[:], in_=Wt[:])
            nc.sync.dma_start(out=b_t[:], in_=bt[:])
            nc.sync.dma_start(out=sdeg_t[:], in_=sdeg[:])
            nc.sync.dma_start(out=dinv_t[:], in_=dinv[:])
            if last_layer:
                boff_t = cp.tile([P, tiles], BF16)
                nc.sync.dma_start(out=boff_t[:], in_=batchoff[:])

            # Per-(group, half) gathers, split into ops of <= MAX_OP_CHUNKS
            # chunks, round-robined across the SWDGE queues (a single queue
            # stalls its descriptor ring on large ops; two queues reach the
            # HBM byte roofline).
            def gather_ops(grp):
                """[(g_chunk_start, n_chunks, idx_chunk_start, table_half)]"""
                ops = []
                Lg, Hg = grp['Lg'], grp['Hg']
                co = grp['chunk_off']
                for half, base, n in ((0, 0, Lg), (1, Lg, Hg)):
                    c = 0
                    while c < n:
                        cn = min(MAX_OP_CHUNKS, n - c)
                        ops.append((base + c, cn, co + base + c, half))
                        c += cn
                return ops

            reg_cache = {}
            for grp in groups:
                for (_, cn, _, _) in gather_ops(grp):
                    if cn * P not in reg_cache:
                        reg_cache[cn * P] = nc.gpsimd.to_reg(cn * P)

            qrr = 0
            for rep in range(reps):
                if last_layer:
                    pool_acc = pp3.tile([P, D], F32, space="PSUM", tag="pool_acc")
                for gi, grp in enumerate(groups):
                    ts = grp['tiles']
                    Lg, Hg = grp['Lg'], grp['Hg']
                    g = gp.tile([P, cg_max * D], TDT, tag="g")
                    for (gc0, cn, ic0, half) in gather_ops(grp):
                        src = table[0:H, :] if half == 0 else table[H:npad, :]
                        nc.gpsimd.dma_gather(
                            g[:, gc0 * D:(gc0 + cn) * D]
                                .rearrange("p (c j) -> p c j", j=D),
                            src,
                            idx_t[:, ic0 * 8:(ic0 + cn) * 8],
                            cn * P, reg_cache[cn * P], D,
                            single_packet=False, queue_num=qrr % NQ)
                        qrr += 1
                    if not last_layer:
                        hh_g = hp.tile([P, len(ts) * D], TDT, tag="hh")
                    for ti, t in enumerate(ts):
                        nlo, nhi = int(c_lo[t]), int(c_hi[t])
                        ct = nlo + nhi
                        col = int(tile_col[t])
                        S = sp.tile([P, ct_max * D], BF16, tag="s")
                        nc.vector.tensor_tensor(
                            out=S[:, :ct * D].rearrange("p (c j) -> p c j", j=D),
                            in0=dst_t[:, col:col + ct].to_broadcast([P, ct, D]),
                            in1=iota_t[:, :ct * D].rearrange("p (c j) -> p c j", j=D),
                            op=mybir.AluOpType.is_equal)
                        psumT = pp1.tile([P, D], F32, space="PSUM", tag="pT")
                        gchunks = ([grp['lo_start'][t] + c for c in range(nlo)] +
                                   [Lg + grp['hi_start'][t] + c for c in range(nhi)])
                        for ci, gc in enumerate(gchunks):
                            nc.tensor.matmul(out=psumT[:],
                                             lhsT=g[:, gc * D:(gc + 1) * D],
                                             rhs=S[:, ci * D:(ci + 1) * D],
                                             start=(ci == 0), stop=(ci == ct - 1))
                        lhs_sb = mp.tile([P, D], F32, tag="lhs")
                        nc.scalar.activation(out=lhs_sb[:], in_=psumT[:],
                                             func=mybir.ActivationFunctionType.Copy)
                        psum2 = pp2.tile([P, D], F32, space="PSUM", tag="p2")
                        nc.tensor.matmul(out=psum2[:], lhsT=lhs_sb[:], rhs=W_t[:],
                                         start=True, stop=False)
                        nc.tensor.matmul(out=psum2[:],
                                         lhsT=sdeg_t[:, t * P:(t + 1) * P],
                                         rhs=b_t[:], start=False, stop=True)
                        if last_layer:
                            out_sb = mp.tile([P, D], F32, tag="out")
                            nc.scalar.activation(out=out_sb[:], in_=psum2[:],
                                                 func=mybir.ActivationFunctionType.Copy,
                                                 scale=dinv_t[:, t:t + 1])
                            Pt = sp.tile([P, D], F32, tag="pool_sel")
                            nc.vector.tensor_tensor(
                                out=Pt[:],
                                in0=boff_t[:, t:t + 1].to_broadcast([P, D]),
                                in1=iota_t[:, :D],
                                op=mybir.AluOpType.is_equal)
                            nc.tensor.matmul(out=pool_acc[:], lhsT=Pt[:],
                                             rhs=out_sb[:],
                                             start=(t == 0), stop=(t == tiles - 1))
                        else:
                            out_sb = mp.tile([P, D], F32, tag="out")
                            nc.scalar.activation(out=out_sb[:], in_=psum2[:],
                                                 func=mybir.ActivationFunctionType.Relu,
                                                 scale=dinv_t[:, t:t + 1])
                            nc.vector.tensor_scalar_mul(
                                out=hh_g[:, ti * D:(ti + 1) * D], in0=out_sb[:],
                                scalar1=dinv_t[:, t:t + 1])
                    if not last_layer:
                        t0 = ts[0]
                        nc.sync.dma_start(
                            out=hhat[t0 * P:(t0 + len(ts)) * P, :]
                                .rearrange("(c p) j -> p c j", p=P),
                            in_=hh_g[:, :len(ts) * D].rearrange("p (c j) -> p c j", j=D))
                if last_layer:
                    pooled_sb = mp.tile([P, D], F32, tag="pooled_sb")
                    nc.scalar.activation(out=pooled_sb[:], in_=pool_acc[:],
                                         func=mybir.ActivationFunctionType.Copy)
                    nc.sync.dma_start(out=pooled[:], in_=pooled_sb[:])
    nc.compile()
    return nc


def _in_maps(pre, table_np, W, b, last_layer):
    maps = []
    for k in range(N_CORES):
        m = dict(table=table_np,
                 idx=pre['idx_w'][k],
                 dstoff=pre['dstoff'][k].astype(ml_dtypes.bfloat16),
                 iota=pre['iota'].astype(ml_dtypes.bfloat16),
                 W=np.ascontiguousarray(W, dtype=np.float32),
                 b=np.ascontiguousarray(b, dtype=np.float32).reshape(1, D),
                 dinv=pre['dinv_slab'][k],
                 sdeg=pre['sdeg_rows'][k])
        if last_layer:
            m['batchoff'] = pre['batchoff'][k].astype(ml_dtypes.bfloat16)
        maps.append(m)
    return maps


def kernel(x, edge_index, batch, W1, b1, W2, b2):
    x = np.asarray(x); edge_index = np.asarray(edge_index)
    batch = np.asarray(batch)
    W1 = np.asarray(W1); b1 = np.asarray(b1)
    W2 = np.asarray(W2); b2 = np.asarray(b2)

    pre = preprocess(x, edge_index, batch)
    core_ids = list(range(N_CORES))

    tdt = ml_dtypes.bfloat16 if USE_BF16 else np.float32
    table1 = pre['xhat'].astype(tdt)
    nc1 = build_layer(pre, last_layer=False, bf16_table=USE_BF16)
    res1 = run_bass_kernel_spmd(nc1, _in_maps(pre, table1, W1, b1, False),
                                core_ids).results

    h1hat = np.zeros((pre['npad'], D), dtype=tdt)
    rpc = pre['rows_per_core']
    for k in range(N_CORES):
        h1hat[k * rpc:(k + 1) * rpc] = res1[k]['hhat']

    nc2 = build_layer(pre, last_layer=True, bf16_table=USE_BF16)
    res2 = run_bass_kernel_spmd(nc2, _in_maps(pre, h1hat, W2, b2, True),
                                core_ids).results

    pooled = np.zeros((G, D), dtype=np.float32)
    for k in range(N_CORES):
        part = res2[k]['pooled']
        g0 = int(pre['g0'][k])
        span = min(P, G - g0)
        pooled[g0:g0 + span] += part[:span]
    return pooled / np.maximum(pre['cnt_g'], 1.0)[:, None]
